# revision 2
# baseline (speedup 1.0000x reference)
"""BertScore model kernel for Trainium2 (8 NeuronCores, SPMD data-parallel over B).

Reference computation: cosine-normalized per-layer token reps, per-(layer,batch)
similarity matrix dots = h1 @ h2^T (256x256, contraction D=1024), ragged masked
max over rows/cols + masked means -> s1,s2, F1 harmonic mean -> (B,NL) features,
BatchNorm over batch, linear head -> (B,).

v2 vs the fp16 baseline (295 us measured / 124 us DMA-only measured):
- fp8 e4m3 inputs (h scaled by 32; |h|<=1 so max 32 << 240 TRN e4m3 limit)
  with DoubleRow matmuls: half the DMA bytes, half the PE matmul cycles.
- Host planar layout (128, B, 2, NL, KT, L): each SBUF partition reads ONE
  contiguous 16KB*u run per DMA (vs 4KB runs before) - the measured HW DMA
  inefficiency (258 GB/s vs 358 peak) was descriptor/run-length bound.
- bf16 intermediate sim matrix (ACT PSUM->SBUF copy, PE transpose, DVE
  reduces) for 2x DVE throughput; reduces batched over both 128-halves.

Masks are applied additively (0 valid / -1e30 invalid): m2[j] via a K=1 f32r
matmul into the PSUM accumulation group, m1[i] via the ACT bias during the
PSUM->SBUF copy (constant along the reduced axis, so row maxes are unaffected
for valid rows; invalid rows/cols are dropped in the host epilogue).
"""
import os
import numpy as np

NL, B, L1, L2, D = 4, 64, 256, 256, 1024
NCORES = 8
BB = B // NCORES          # batches per core
KT = D // 128             # contraction subtiles
NEG = -1.0e30             # additive mask for invalid positions
SCALE = 32.0              # fp8 input scale; dots come back scaled by SCALE**2
BN_EPS = 1e-8
LOGIT_SCALE = 1.0

DTYPE = os.environ.get("BSM_DTYPE", "f8")        # f8 | f16
REPEAT = int(os.environ.get("BSM_REPEAT", "1"))  # body repeats (for timing)
U = int(os.environ.get("BSM_U", "2"))            # batches merged per DMA
SKIP = set(os.environ.get("BSM_SKIP", "").split(","))  # debug: mm,act,red,dt
IOBUFS = int(os.environ.get("BSM_IOBUFS", "2"))
LOOPN = int(os.environ.get("BSM_LOOPN", "0"))  # >0: wrap body in device For_i loop

_CACHE = {}


def _build(dtype_name, repeat, u, iobufs):
    import concourse.bacc as bacc
    import concourse.bass as bass
    import concourse.mybir as mybir
    import concourse.tile as tile
    from concourse.masks import make_identity

    f32 = mybir.dt.float32
    f32r = mybir.dt.float32r
    bf16 = mybir.dt.bfloat16
    dt_in = {"f8": mybir.dt.float8e4, "f16": mybir.dt.float16}[dtype_name]
    fp8 = dt_in == mybir.dt.float8e4

    nc = bacc.Bacc("TRN2", target_bir_lowering=False, debug=False,
                   num_devices=NCORES)

    # combined planar input: hb[p, b, t, l, q, i] = h_t[l, b, i, q*128+p]
    hbd = nc.dram_tensor("hb", [128, BB, 2, NL, KT, L1], dt_in,
                         kind="ExternalInput")
    # m1 as per-partition columns: m1c[p,b,h] = m1[b, h*128+p]
    m1c = nc.dram_tensor("m1c", [128, BB, 2], f32, kind="ExternalInput")
    m2d = nc.dram_tensor("m2", [BB, L2], f32r, kind="ExternalInput")
    onesd = nc.dram_tensor("ones", [1, 128], f32r, kind="ExternalInput")
    NCOL = NL * BB * 2
    rmd = nc.dram_tensor("rm", [128, NCOL], f32, kind="ExternalOutput")
    cmd = nc.dram_tensor("cm", [128, NCOL], f32, kind="ExternalOutput")

    with tile.TileContext(nc) as tc:
        with tc.tile_pool(name="consts", bufs=1) as consts, \
             tc.tile_pool(name="io", bufs=iobufs) as io, \
             tc.tile_pool(name="dsbp", bufs=4) as dsbp, \
             tc.tile_pool(name="accp", bufs=1) as accp, \
             tc.tile_pool(name="ps", bufs=3, space="PSUM") as ps, \
             tc.tile_pool(name="psT", bufs=2, space="PSUM") as psT:

            ident = consts.tile([128, 128], bf16)
            make_identity(nc, ident)
            ones = consts.tile([1, 128], f32r)
            nc.sync.dma_start(out=ones, in_=onesd.ap())

            m2sb = consts.tile([1, BB, L2], f32r)
            m2ap = m2d.ap()
            nc.sync.dma_start(out=m2sb, in_=bass.AP(
                tensor=m2ap.tensor, offset=m2ap.offset,
                ap=[[0, 1], [L2, BB], [1, L2]]))
            m1sb = consts.tile([128, BB, 2], f32)
            nc.sync.dma_start(out=m1sb, in_=m1c.ap())

            RM = accp.tile([128, NCOL], f32)
            CM = accp.tile([128, NCOL], f32)
            if SKIP & {"mm", "act", "red", "dt"}:
                nc.vector.memset(RM, 0.0)
                nc.vector.memset(CM, 0.0)

            hbap = hbd.ap()
            vmax = mybir.AluOpType.max
            X = mybir.AxisListType.X
            IDENT = mybir.ActivationFunctionType.Identity
            DR = mybir.MatmulPerfMode.DoubleRow

            import contextlib
            loop_cm = (tc.For_i(0, LOOPN, 1,
                                hint_engines=(mybir.EngineType.PE,))
                       if LOOPN > 0 else contextlib.nullcontext())
            with loop_cm:
              for _rep in range(repeat):
                for bu in range(BB // u):
                    hbt = io.tile([128, u, 2, NL, KT, L1], dt_in, tag="hb")
                    nc.sync.dma_start(
                        out=hbt, in_=hbap[:, bu * u:(bu + 1) * u])
                    for ul in range(u):
                        b = bu * u + ul
                        for l in range(NL):
                            if "mm" in SKIP:
                                continue
                            dps = ps.tile([128, 2, L2], f32, tag="dots")
                            for it in range(2):
                                i0 = it * 128
                                if fp8:
                                    for qp in range(0, KT, 2):
                                        nc.tensor.matmul(
                                            out=dps[:, it, :],
                                            lhsT=hbt[:, ul, 0, l, qp:qp + 2,
                                                     i0:i0 + 128],
                                            rhs=hbt[:, ul, 1, l, qp:qp + 2, :],
                                            start=(qp == 0), stop=False,
                                            perf_mode=DR)
                                else:
                                    for k in range(KT):
                                        nc.tensor.matmul(
                                            out=dps[:, it, :],
                                            lhsT=hbt[:, ul, 0, l, k,
                                                     i0:i0 + 128],
                                            rhs=hbt[:, ul, 1, l, k, :],
                                            start=(k == 0), stop=False)
                                # += m2[j] on every row (K=1 accumulate)
                                nc.tensor.matmul(out=dps[:, it, :], lhsT=ones,
                                                 rhs=m2sb[:, b, :],
                                                 start=False, stop=True)
                            if "act" in SKIP:
                                continue
                            # PSUM->SBUF copy in bf16 with per-partition
                            # m1[i] added (ACT bias)
                            dsb = dsbp.tile([128, 2, L2], bf16, tag="dsb")
                            for it in range(2):
                                nc.scalar.activation(
                                    out=dsb[:, it, :], in_=dps[:, it, :],
                                    func=IDENT, bias=m1sb[:, b, it:it + 1])
                            col = (l * BB + b) * 2
                            if "red" not in SKIP:
                                # row max over j for both 128-halves at once
                                nc.vector.tensor_reduce(
                                    out=RM[:, col:col + 2], in_=dsb,
                                    axis=X, op=vmax)
                            if "dt" in SKIP:
                                continue
                            dT = psT.tile([128, 2, L1], bf16, tag="dT")
                            for jt in range(2):
                                for it in range(2):
                                    nc.tensor.transpose(
                                        out=dT[:, jt, it * 128:(it + 1) * 128],
                                        in_=dsb[:, it,
                                                jt * 128:(jt + 1) * 128],
                                        identity=ident)
                            if "red" not in SKIP:
                                nc.vector.tensor_reduce(
                                    out=CM[:, col:col + 2], in_=dT,
                                    axis=X, op=vmax)

            nc.sync.dma_start(out=rmd.ap(), in_=RM)
            nc.sync.dma_start(out=cmd.ap(), in_=CM)

    nc.finalize()
    return nc


def _get_nc():
    key = (DTYPE, REPEAT, U, IOBUFS, LOOPN, tuple(sorted(SKIP)))
    if key not in _CACHE:
        _CACHE[key] = _build(*key[:4])
    return _CACHE[key]


def _host_prep(reps1, reps2, len1, len2):
    """Normalize+scale, pack to planar fp8 (128, B, 2, NL, KT, L); masks."""
    import ml_dtypes
    np_in = {"f8": ml_dtypes.float8_e4m3, "f16": np.float16}[DTYPE]

    def planar(r):
        r = np.asarray(r, dtype=np.float32)
        n = np.sqrt(np.einsum('lbid,lbid->lbi', r, r))
        h = r * (SCALE / n[..., None])                # (NL, B, L, D)
        x = h.reshape(NL, B, L1, KT, 128)             # d = q*128 + p
        return x.transpose(4, 1, 0, 3, 2).astype(np_in)   # (128, B, NL, KT, L)

    hb = np.stack([planar(reps1), planar(reps2)], axis=2)  # (128,B,2,NL,KT,L)
    len1 = np.asarray(len1).astype(np.int64)
    len2 = np.asarray(len2).astype(np.int64)
    ar1 = np.arange(L1)[None, :]
    ar2 = np.arange(L2)[None, :]
    m1 = np.where(ar1 < len1[:, None], 0.0, NEG).astype(np.float32)  # (B, L1)
    m2 = np.where(ar2 < len2[:, None], 0.0, NEG).astype(np.float32)
    # (B, L1) -> (B, 2, 128) -> (128, B, 2)
    m1c = np.ascontiguousarray(m1.reshape(B, 2, 128).transpose(2, 0, 1))

    in_maps = []
    for c in range(NCORES):
        sl = slice(c * BB, (c + 1) * BB)
        in_maps.append({
            "hb": np.ascontiguousarray(hb[:, sl]),
            "m1c": np.ascontiguousarray(m1c[:, sl]),
            "m2": np.ascontiguousarray(m2[sl]),
            "ones": np.ones((1, 128), dtype=np.float32),
        })
    return in_maps, len1, len2


def _epilogue(results, len1, len2, w, b):
    """rm/cm (128, NL*BB*2) per core -> s1,s2 -> F1 -> BatchNorm -> head."""
    maxv_rows = np.empty((NL, B, L1), dtype=np.float64)  # max over valid j, per i
    maxv_cols = np.empty((NL, B, L2), dtype=np.float64)  # max over valid i, per j
    for c, res in enumerate(results):
        rm = np.asarray(res["rm"], dtype=np.float64)  # (128, NCOL)
        cm = np.asarray(res["cm"], dtype=np.float64)
        # column t = (l*BB + b)*2 + half ; partition p -> index half*128 + p
        rm_r = rm.T.reshape(NL, BB, 2, 128).reshape(NL, BB, 256)
        cm_r = cm.T.reshape(NL, BB, 2, 128).reshape(NL, BB, 256)
        maxv_rows[:, c * BB:(c + 1) * BB] = rm_r
        maxv_cols[:, c * BB:(c + 1) * BB] = cm_r
    inv = 1.0 / (SCALE * SCALE)
    maxv_rows *= inv
    maxv_cols *= inv

    ar1 = np.arange(L1)[None, :]
    ar2 = np.arange(L2)[None, :]
    mask1 = (ar1 < len1[:, None])  # (B, L1)
    mask2 = (ar2 < len2[:, None])
    n1 = len1.astype(np.float64)
    n2 = len2.astype(np.float64)

    # s2: mean over valid i of (max over valid j); s1: mean over valid j of
    # (max over valid i)
    s2 = np.where(mask1[None], maxv_rows, 0.0).sum(axis=2) / n1[None]  # (NL, B)
    s1 = np.where(mask2[None], maxv_cols, 0.0).sum(axis=2) / n2[None]
    feat = (2.0 * s1 * s2 / (s1 + s2)).T                    # (B, NL)
    mean = feat.mean(axis=0, keepdims=True)
    var = ((feat - mean) ** 2).mean(axis=0, keepdims=True)
    feat = (feat - mean) / np.sqrt(var + BN_EPS)
    w = np.asarray(w, dtype=np.float64)
    bb = np.asarray(b, dtype=np.float64)
    out = LOGIT_SCALE * (feat @ w.T + bb)[:, 0]
    return out.astype(np.float32)


LAST_RUN = {}


def kernel(reps1, reps2, len1, len2, w, b):
    from concourse.bass_utils import run_bass_kernel_spmd

    nc = _get_nc()
    in_maps, l1, l2 = _host_prep(reps1, reps2, len1, len2)
    res = run_bass_kernel_spmd(nc, in_maps, list(range(NCORES)))
    LAST_RUN["results"] = res
    LAST_RUN["in_maps"] = in_maps
    return _epilogue(res.results, l1, l2, w, b)


# revision 5
# speedup vs baseline: 1.6404x; 1.6404x over previous
"""BertScore model kernel for Trainium2 (8 NeuronCores, SPMD data-parallel over B).

Reference computation: cosine-normalized per-layer token reps, per-(layer,batch)
similarity matrix dots = h1 @ h2^T (256x256, contraction D=1024), ragged masked
max over rows/cols + masked means -> s1,s2, F1 harmonic mean -> (B,NL) features,
BatchNorm over batch, linear head -> (B,).

v3 (ragged) on top of the fp8 v2:
- fp8 e4m3 inputs (h scaled by 32; |h|<=1 so max 32 << 240 TRN e4m3 limit)
  with DoubleRow matmuls: half the DMA bytes, half the PE matmul cycles.
- Host planar layout: each SBUF partition reads ONE contiguous run per DMA
  (the measured HW DMA inefficiency was descriptor/run-length bound).
- bf16 intermediate sim matrix (ACT PSUM->SBUF copy, PE transpose, DVE
  reduces); reduces batched over the 128-halves.
- Ragged: the 64 batches are clustered into 8 SPMD slots (one batch per
  core per slot) sized to the cluster maxima (I_k, J_k); DMA, matmuls and
  reduces only touch [0:I_k] x [0:J_k] (~69% of the dense volume for the
  reference length distribution). The compiled program depends on the
  length arrays; builds are cached per slot-size tuple, so new length sets
  recompile but stay correct.

Masks are applied additively (0 valid / -1e30 invalid) and cover
[len, slotmax): m2[j] via a K=1 f32r matmul into the PSUM accumulation
group, m1[i] via the ACT bias during the PSUM->SBUF copy (constant along
the reduced axis, so row maxes are unaffected for valid rows; invalid
rows/cols are dropped in the host epilogue).
"""
import os
import numpy as np

NL, B, L1, L2, D = 4, 64, 256, 256, 1024
NCORES = 8
BB = B // NCORES          # batch slots per core
KT = D // 128             # contraction subtiles
RW = 2 * NL * KT          # bytes per (partition, token-row) in the planar pack
NEG = -1.0e30             # additive mask for invalid positions
SCALE = 32.0              # fp8 input scale; dots come back scaled by SCALE**2
BN_EPS = 1e-8
LOGIT_SCALE = 1.0

DTYPE = os.environ.get("BSM_DTYPE", "f8")        # f8 | f16
REPEAT = int(os.environ.get("BSM_REPEAT", "1"))  # body repeats (for timing)
DENSE = int(os.environ.get("BSM_DENSE", "0"))    # 1: pad all slots to 256
SKIP = set(os.environ.get("BSM_SKIP", "").split(","))  # debug: io,mm,act,red,dt
IOBUFS = int(os.environ.get("BSM_IOBUFS", "3"))
LOOPN = int(os.environ.get("BSM_LOOPN", "0"))  # >0: wrap body in device For_i loop

_CACHE = {}


def _build(dtype_name, repeat, iobufs, slots):
    """slots: tuple of (I_k, J_k) compile-time sizes for the BB batch slots."""
    import concourse.bacc as bacc
    import concourse.bass as bass
    import concourse.mybir as mybir
    import concourse.tile as tile
    from concourse.masks import make_identity

    f32 = mybir.dt.float32
    f32r = mybir.dt.float32r
    bf16 = mybir.dt.bfloat16
    dt_in = {"f8": mybir.dt.float8e4, "f16": mybir.dt.float16}[dtype_name]
    fp8 = dt_in == mybir.dt.float8e4

    nc = bacc.Bacc("TRN2", target_bir_lowering=False, debug=False,
                   num_devices=NCORES)

    # planar ragged pack, per partition p (contiguous, slot-major):
    #   [slot0: h1 (NL,KT,I_0) | h2 (NL,KT,J_0)][slot1: ...] ...
    # where element (t,l,q,i) of slot k is h_t[l, b_k, i, q*128+p] * SCALE
    offs = []     # per-slot (h1_off, h2_off) element offsets
    W = 0
    for (I, J) in slots:
        offs.append((W, W + NL * KT * I))
        W += NL * KT * (I + J)
    hbd = nc.dram_tensor("hb", [128, W], dt_in, kind="ExternalInput")
    # m1 as per-partition columns: m1c[p,k,h] = m1[b_k, h*128+p]
    m1c = nc.dram_tensor("m1c", [128, BB, 2], f32, kind="ExternalInput")
    m2d = nc.dram_tensor("m2", [BB, L2], f32r, kind="ExternalInput")
    onesd = nc.dram_tensor("ones", [1, 128], f32r, kind="ExternalInput")
    NCOL = NL * BB * 2
    rmd = nc.dram_tensor("rm", [128, NCOL], f32, kind="ExternalOutput")
    cmd = nc.dram_tensor("cm", [128, NCOL], f32, kind="ExternalOutput")

    with tile.TileContext(nc) as tc:
        with tc.tile_pool(name="consts", bufs=1) as consts, \
             tc.tile_pool(name="io", bufs=iobufs) as io, \
             tc.tile_pool(name="dsbp", bufs=4) as dsbp, \
             tc.tile_pool(name="accp", bufs=1) as accp, \
             tc.tile_pool(name="ps", bufs=3, space="PSUM") as ps, \
             tc.tile_pool(name="psT", bufs=2, space="PSUM") as psT:

            ident = consts.tile([128, 128], bf16)
            make_identity(nc, ident)
            ones = consts.tile([1, 128], f32r)
            nc.sync.dma_start(out=ones, in_=onesd.ap())

            m2sb = consts.tile([1, BB, L2], f32r)
            m2ap = m2d.ap()
            nc.sync.dma_start(out=m2sb, in_=bass.AP(
                tensor=m2ap.tensor, offset=m2ap.offset,
                ap=[[0, 1], [L2, BB], [1, L2]]))
            m1sb = consts.tile([128, BB, 2], f32)
            nc.sync.dma_start(out=m1sb, in_=m1c.ap())

            RM = accp.tile([128, NCOL], f32)
            CM = accp.tile([128, NCOL], f32)
            if SKIP & {"io", "mm", "act", "red", "dt"}:
                nc.vector.memset(RM, 0.0)
                nc.vector.memset(CM, 0.0)

            hbap = hbd.ap()
            vmax = mybir.AluOpType.max
            X = mybir.AxisListType.X
            IDENT = mybir.ActivationFunctionType.Identity
            DR = mybir.MatmulPerfMode.DoubleRow

            import contextlib
            loop_cm = (tc.For_i(0, LOOPN, 1,
                                hint_engines=(mybir.EngineType.PE,))
                       if LOOPN > 0 else contextlib.nullcontext())
            with loop_cm:
              for _rep in range(repeat):
                for k, (I, J) in enumerate(slots):
                    WK = NL * KT * (I + J)
                    hbt = io.tile([128, WK], dt_in, tag="hb")
                    if "io" not in SKIP:
                        o = offs[k][0]
                        nc.sync.dma_start(out=hbt, in_=hbap[:, o:o + WK])
                    h1v = hbt[:, :NL * KT * I].rearrange(
                        "p (l q i) -> p l q i", l=NL, q=KT)
                    h2v = hbt[:, NL * KT * I:].rearrange(
                        "p (l q j) -> p l q j", l=NL, q=KT)
                    ich = [min(128, I)] + ([I - 128] if I > 128 else [])
                    jch = [min(128, J)] + ([J - 128] if J > 128 else [])
                    for l in range(NL):
                        if "mm" in SKIP:
                            continue
                        dps = ps.tile([128, 2, L2], f32, tag="dots")
                        for it, ci in enumerate(ich):
                            i0 = it * 128
                            if fp8:
                                for qp in range(0, KT, 2):
                                    nc.tensor.matmul(
                                        out=dps[:ci, it, :J],
                                        lhsT=h1v[:, l, qp:qp + 2, i0:i0 + ci],
                                        rhs=h2v[:, l, qp:qp + 2, :],
                                        start=(qp == 0), stop=False,
                                        perf_mode=DR)
                            else:
                                for q in range(KT):
                                    nc.tensor.matmul(
                                        out=dps[:ci, it, :J],
                                        lhsT=h1v[:, l, q, i0:i0 + ci],
                                        rhs=h2v[:, l, q, :],
                                        start=(q == 0), stop=False)
                            # += m2[j] on every row (K=1 accumulate)
                            nc.tensor.matmul(out=dps[:ci, it, :J],
                                             lhsT=ones[:, :ci],
                                             rhs=m2sb[:, k, :J],
                                             start=False, stop=True)
                        if "act" in SKIP:
                            continue
                        # PSUM->SBUF copy in bf16 with m1[i] added (ACT bias)
                        dsb = dsbp.tile([128, 2, L2], bf16, tag="dsb")
                        for it, ci in enumerate(ich):
                            nc.scalar.activation(
                                out=dsb[:ci, it, :J], in_=dps[:ci, it, :J],
                                func=IDENT, bias=m1sb[:ci, k, it:it + 1])
                        col = (l * BB + k) * 2
                        if "red" not in SKIP:
                            # row max over j, both 128-halves of i at once
                            nc.vector.tensor_reduce(
                                out=RM[:, col:col + len(ich)],
                                in_=dsb[:, :len(ich), :J], axis=X, op=vmax)
                        if "dt" in SKIP:
                            continue
                        dT = psT.tile([128, 2, L1], bf16, tag="dT")
                        for jt, cj in enumerate(jch):
                            for it, ci in enumerate(ich):
                                i0 = it * 128
                                nc.tensor.transpose(
                                    out=dT[:cj, jt, i0:i0 + ci],
                                    in_=dsb[:ci, it, jt * 128:jt * 128 + cj],
                                    identity=ident[:ci, :ci])
                        if "red" not in SKIP:
                            nc.vector.tensor_reduce(
                                out=CM[:, col:col + len(jch)],
                                in_=dT[:, :len(jch), :I], axis=X, op=vmax)

            nc.sync.dma_start(out=rmd.ap(), in_=RM)
            nc.sync.dma_start(out=cmd.ap(), in_=CM)

    nc.finalize()
    return nc


def _assign_slots(len1, len2):
    """Cluster the B batches into BB slots of NCORES members, minimizing
    sum over slots of (max len1 + max len2). Returns (perm, slots):
    perm[k][c] = original batch index at (core c, slot k)."""
    import itertools
    l1 = np.asarray(len1).astype(int)
    l2 = np.asarray(len2).astype(int)
    if DENSE:
        perm = [[k * NCORES + c for c in range(NCORES)] for k in range(BB)]
        return perm, [(L1, L2)] * BB
    order = np.argsort(-(l1 + l2))
    groups = [list(order[NCORES * g:NCORES * (g + 1)]) for g in range(BB)]

    def gcost(g):
        return l1[g].max() + l2[g].max()

    best = sum(gcost(g) for g in groups)
    improved = True
    while improved:
        improved = False
        for ga, gb in itertools.combinations(range(BB), 2):
            ca, cb = gcost(groups[ga]), gcost(groups[gb])
            for i in range(NCORES):
                for j in range(NCORES):
                    groups[ga][i], groups[gb][j] = groups[gb][j], groups[ga][i]
                    c = gcost(groups[ga]) + gcost(groups[gb])
                    if c < ca + cb - 1e-9:
                        ca, cb = gcost(groups[ga]), gcost(groups[gb])
                        improved = True
                    else:
                        groups[ga][i], groups[gb][j] = \
                            groups[gb][j], groups[ga][i]
    slots = [(int(l1[g].max()), int(l2[g].max())) for g in groups]
    return [list(map(int, g)) for g in groups], slots


def _get_nc(slots):
    key = (DTYPE, REPEAT, IOBUFS, LOOPN, tuple(sorted(SKIP)), tuple(slots))
    if key not in _CACHE:
        _CACHE[key] = _build(DTYPE, REPEAT, IOBUFS, tuple(slots))
    return _CACHE[key]


def _host_prep(reps1, reps2, len1, len2, perm, slots):
    """Normalize+scale, pack the ragged planar fp8 array per core; masks."""
    import ml_dtypes
    np_in = {"f8": ml_dtypes.float8_e4m3, "f16": np.float16}[DTYPE]

    def planar(r):
        r = np.asarray(r, dtype=np.float32)
        n = np.sqrt(np.einsum('lbid,lbid->lbi', r, r))
        h = r * (SCALE / n[..., None])                # (NL, B, L, D)
        x = h.reshape(NL, B, L1, KT, 128)             # d = q*128 + p
        return x.transpose(4, 1, 0, 3, 2).astype(np_in)   # (128, B, NL, KT, L)

    p1 = planar(reps1)
    p2 = planar(reps2)
    len1 = np.asarray(len1).astype(np.int64)
    len2 = np.asarray(len2).astype(np.int64)
    ar1 = np.arange(L1)[None, :]
    ar2 = np.arange(L2)[None, :]
    m1 = np.where(ar1 < len1[:, None], 0.0, NEG).astype(np.float32)  # (B, L1)
    m2 = np.where(ar2 < len2[:, None], 0.0, NEG).astype(np.float32)

    W = sum(NL * KT * (I + J) for (I, J) in slots)
    in_maps = []
    for c in range(NCORES):
        hb = np.empty((128, W), dtype=np_in)
        m1ck = np.empty((128, BB, 2), dtype=np.float32)
        m2k = np.empty((BB, L2), dtype=np.float32)
        o = 0
        for k, (I, J) in enumerate(slots):
            b = perm[k][c]
            # h1 rows [0:I]: (NL, KT, I) slab, i-minor
            n1 = NL * KT * I
            hb[:, o:o + n1] = p1[:, b, :, :, :I].reshape(128, n1)
            n2 = NL * KT * J
            hb[:, o + n1:o + n1 + n2] = p2[:, b, :, :, :J].reshape(128, n2)
            o += n1 + n2
            m1ck[:, k, :] = m1[b].reshape(2, 128).T
            m2k[k] = m2[b]
        in_maps.append({
            "hb": hb,
            "m1c": m1ck,
            "m2": m2k,
            "ones": np.ones((1, 128), dtype=np.float32),
        })
    return in_maps, len1, len2


def _epilogue(results, len1, len2, w, b, perm):
    """rm/cm (128, NL*BB*2) per core -> s1,s2 -> F1 -> BatchNorm -> head."""
    maxv_rows = np.empty((NL, B, L1), dtype=np.float64)  # max over valid j, per i
    maxv_cols = np.empty((NL, B, L2), dtype=np.float64)  # max over valid i, per j
    for c, res in enumerate(results):
        rm = np.asarray(res["rm"], dtype=np.float64)  # (128, NCOL)
        cm = np.asarray(res["cm"], dtype=np.float64)
        # column t = (l*BB + k)*2 + half ; partition p -> index half*128 + p
        rm_r = rm.T.reshape(NL, BB, 2, 128).reshape(NL, BB, 256)
        cm_r = cm.T.reshape(NL, BB, 2, 128).reshape(NL, BB, 256)
        for k in range(BB):
            bidx = perm[k][c]
            maxv_rows[:, bidx] = rm_r[:, k]
            maxv_cols[:, bidx] = cm_r[:, k]
    inv = 1.0 / (SCALE * SCALE)
    maxv_rows *= inv
    maxv_cols *= inv

    ar1 = np.arange(L1)[None, :]
    ar2 = np.arange(L2)[None, :]
    mask1 = (ar1 < len1[:, None])  # (B, L1)
    mask2 = (ar2 < len2[:, None])
    n1 = len1.astype(np.float64)
    n2 = len2.astype(np.float64)

    # s2: mean over valid i of (max over valid j); s1: mean over valid j of
    # (max over valid i)
    with np.errstate(invalid="ignore"):
        s2 = np.where(mask1[None], maxv_rows, 0.0).sum(axis=2) / n1[None]
        s1 = np.where(mask2[None], maxv_cols, 0.0).sum(axis=2) / n2[None]
    feat = (2.0 * s1 * s2 / (s1 + s2)).T                    # (B, NL)
    mean = feat.mean(axis=0, keepdims=True)
    var = ((feat - mean) ** 2).mean(axis=0, keepdims=True)
    feat = (feat - mean) / np.sqrt(var + BN_EPS)
    w = np.asarray(w, dtype=np.float64)
    bb = np.asarray(b, dtype=np.float64)
    out = LOGIT_SCALE * (feat @ w.T + bb)[:, 0]
    return out.astype(np.float32)


LAST_RUN = {}


def kernel(reps1, reps2, len1, len2, w, b):
    from concourse.bass_utils import run_bass_kernel_spmd

    perm, slots = _assign_slots(len1, len2)
    nc = _get_nc(slots)
    in_maps, l1, l2 = _host_prep(reps1, reps2, len1, len2, perm, slots)
    res = run_bass_kernel_spmd(nc, in_maps, list(range(NCORES)))
    LAST_RUN["results"] = res
    LAST_RUN["in_maps"] = in_maps
    LAST_RUN["nc"] = nc
    LAST_RUN["slots"] = slots
    return _epilogue(res.results, l1, l2, w, b, perm)
